# revision 25
# baseline (speedup 1.0000x reference)
"""Trainium2 Bass kernel for nn_ExpertMLP (MoE routing).

Strategy (tensor-parallel over d_ff, host-side dispatch):
  - Every core processes ALL T token-expert pairs (the concatenation of
    each expert's distinct hit tokens, duplicate top-k hits merged with
    summed gate weight), but only a 512-wide slice of the FFN dim F:
    core j holds w1[e, j*512:(j+1)*512, :] and w2[e, :, j*512:(j+1)*512]
    for all 8 experts (16 MB bf16 -- same footprint as one full expert).
  - silu is elementwise, so the F-slice passes through it exactly:
        part_j = silu(x @ W1_j.T) @ W2_j.T          # [T, H]
        y(pair) = wc(pair) * sum_j part_j(pair)
    The host sums the 8 bf16 partials, applies wc, and scatter-adds per
    expert segment into the full [S, H] output.
  - Work per core is T * 2*2*H*(F/8) MACs -- perfectly balanced by
    construction (no capacity padding, no expert imbalance), ~6% less
    than the expert-parallel layout's max-expert padding, and with the
    H-major second GEMM there is no partial-tile waste at all:
    the PE stream is exactly 64*T cycles.

Device kernel layout (per core):
  The pair stream is cut into per-expert chunks of <=512 tokens, so every
  chunk uses a single expert's (resident) weight slice. Per chunk:
    phase 1:  for f in 4:  ph[128, w] += w1_sb[e][h, f-tile].T @ x[h]   (8 MMs)
              silu(ph) -> hs[:, f, :]   (ACT engine)
    gemm 2:   for h in 8:  pyh[128, w] += w2_sb[e][f, h-tile].T @ hs[f]  (4 MMs)
              copy pyh -> y_sb[:, h, :]  (DVE, f32->bf16)
    one [128, 8, w] bf16 DMA out per chunk (y stays H-major; the host
    transposes once after summing cores).
  All DMAs ride the sync-engine HWDGE queue in consumption order, one
  enqueue per chunk / per weight tensor (the host pre-lays p-major
  layouts so each load is a single 3D access pattern). Expert k's
  weights are enqueued three chunks before their first use. The scalar
  engine runs ONLY activations (its table is preloaded during the PE
  warm-up) so silu is never stuck behind DMA enqueues.

  SBUF/partition: w1 64KB + w2 64KB + x 32KB + hs 8KB + y 16KB ~= 184KB.
  PSUM banks: 2 (ph) + 4 (pyh) + 2 (warm) = 8.
"""

import numpy as np
import ml_dtypes

import concourse.bacc as bacc
import concourse.mybir as mybir
import concourse.tile as tile
from concourse.bass_utils import run_bass_kernel_spmd

P = 128
H = 1024
F = 4096
E = 8
N_CORES = 8
CHUNK = 512
FS = F // N_CORES   # 512: per-core f-slice
FTS = FS // P       # 4 f-tiles per core
HT = H // P         # 8

BF16 = mybir.dt.bfloat16
F32 = mybir.dt.float32

# Results of the most recent device run (BassKernelResults); lets a test
# harness read exec_time_ns / trace paths without changing kernel()'s API.
LAST_RESULTS = None

_program_cache = {}


def _make_plan(counts):
    """Cut the concatenated per-expert pair stream into single-expert
    chunks of <=512 tokens. Returns a list of (g0, w, e)."""
    chunks = []
    g0 = 0
    for e, L in enumerate(counts):
        if L == 0:
            continue
        n = -(-L // CHUNK)
        widths = [CHUNK] * (n - 1) + [L - CHUNK * (n - 1)]
        if n >= 2 and widths[-1] < P:
            widths[-2] = CHUNK // 2
            widths[-1] += CHUNK // 2
        for w in widths:
            chunks.append((g0, w, e))
            g0 += w
    # Halve the first chunk: chunk 0's x bytes gate the first matmul.
    if chunks and chunks[0][1] == CHUNK:
        g0, w, e = chunks[0]
        chunks[0:1] = [(g0, w // 2, e), (g0 + w // 2, w - w // 2, e)]
    return chunks


def _build_program(plan_key):
    chunks, T = plan_key
    nc = bacc.Bacc(None, name="expert_mlp_tp")

    xt_d = nc.dram_tensor("xt", (P, HT, T), BF16, kind="ExternalInput")
    w1t_d = nc.dram_tensor("w1t", (E, P, HT, FS), BF16, kind="ExternalInput")
    w2t_d = nc.dram_tensor("w2t", (E, P, FTS, H), BF16, kind="ExternalInput")
    y_d = nc.dram_tensor("y", (P, HT, T), BF16, kind="ExternalOutput")

    silu = mybir.ActivationFunctionType.Silu

    # Emit expert k's weight load 3 chunks ahead of its first chunk.
    first_chunk = {}
    for ci, (_, _, e) in enumerate(chunks):
        first_chunk.setdefault(e, ci)
    w_sched = {}
    head_experts = []
    for e, fc in first_chunk.items():
        at = fc - 3
        if at <= 0:
            head_experts.append(e)
        else:
            w_sched.setdefault(at, []).append(e)

    with tile.TileContext(nc) as tc:
        with (
            tc.tile_pool(name="wpool", bufs=1) as wpool,
            tc.tile_pool(name="xpool", bufs=4) as xpool,
            tc.tile_pool(name="hpool", bufs=2) as hpool,
            tc.tile_pool(name="ypool", bufs=2) as ypool,
            tc.tile_pool(name="spool", bufs=1) as spool,
            tc.tile_pool(name="psh", bufs=3, space="PSUM") as psh,
            tc.tile_pool(name="psy", bufs=1, space="PSUM") as psy,
        ):
            w1_sb = [
                wpool.tile([P, HT, FS], BF16, tag=f"w1_{e}", name=f"w1_{e}")
                for e in range(E)
            ]
            w2_sb = [
                wpool.tile([P, FTS, H], BF16, tag=f"w2_{e}", name=f"w2_{e}")
                for e in range(E)
            ]

            def load_w_expert(e):
                # Steady-state expert loads ride the scalar HWDGE queue --
                # the sync engine's enqueue budget (x + y per chunk) is
                # nearly full, and a weight burst there can delay an x
                # prefetch past its deadline. Emitted AFTER a chunk's four
                # ACTs, so silu never queues behind these.
                nc.scalar.dma_start(w1_sb[e][:], w1t_d[e])
                nc.scalar.dma_start(w2_sb[e][:], w2t_d[e])

            def load_x_chunk(ci, split=1):
                # split>1 fans the load over several DMA rings -- used for
                # chunk 0, whose arrival gates the first real matmul.
                g0, w, _ = chunks[ci]
                t = xpool.tile([P, HT, CHUNK], BF16, tag="x", name="x")[:, :, :w]
                hstep = HT // split
                for s in range(split):
                    h0 = s * hstep
                    nc.sync.dma_start(
                        t[:, h0 : h0 + hstep, :],
                        xt_d[:, h0 : h0 + hstep, g0 : g0 + w],
                    )
                return t

            # Head DMAs, all on the sync queue (scalar-queue enqueues proved
            # slow and block the ACTs behind them). Enqueue instructions
            # cost ~0.6us of engine time each, so the order interleaves the
            # two tensors the first matmul actually waits on: the first
            # half of x chunk 0 and w1's first f-tile quarter. w2 is only
            # needed once gemm 2 starts and goes last.
            g0_0, w_0, _ = chunks[0]
            x_t0 = xpool.tile([P, HT, CHUNK], BF16, tag="x", name="x")[:, :, :w_0]
            hh = HT // 2
            nc.sync.dma_start(x_t0[:, :hh, :], xt_d[:, :hh, g0_0 : g0_0 + w_0])
            e0 = sorted(head_experts)[0] if head_experts else None
            if e0 is not None:
                nc.sync.dma_start(
                    w1_sb[e0][:, :, 0:P], w1t_d[e0][:, :, 0:P]
                )
            nc.sync.dma_start(x_t0[:, hh:, :], xt_d[:, hh:, g0_0 : g0_0 + w_0])
            if e0 is not None:
                for f in range(1, FTS):
                    nc.sync.dma_start(
                        w1_sb[e0][:, :, f * P : (f + 1) * P],
                        w1t_d[e0][:, :, f * P : (f + 1) * P],
                    )
            x_chunks = [x_t0]
            if len(chunks) > 1:
                x_chunks.append(load_x_chunk(1))
            for e in sorted(head_experts):
                if e != e0:
                    nc.sync.dma_start(w1_sb[e][:], w1t_d[e])
                # split by H half: gemm 2's first half-group only needs
                # columns 0:512 (all f), so it can start off the first DMA.
                nc.sync.dma_start(w2_sb[e][:, :, 0 : H // 2], w2t_d[e][:, :, 0 : H // 2])
                nc.sync.dma_start(w2_sb[e][:, :, H // 2 :], w2t_d[e][:, :, H // 2 :])

            # PE pre-warm: dependency-free matmuls on a zeroed scratch tile
            # run while the head DMAs are in flight, so the HAM clock gate
            # is at 8/8 when the real matmuls start. One of them feeds a
            # throwaway silu so the ACT table loads off the critical path.
            warm_sb = spool.tile([P, P], BF16, tag="warm", name="warm_sb")
            warm_act = spool.tile([P, 64], BF16, tag="warm_act", name="warm_act")
            nc.gpsimd.memset(warm_sb[:], 0.0)
            warm_ps = psy.tile([P, 64], F32, tag="warm_ps", name="warm_ps")
            nc.tensor.matmul(warm_ps[:], warm_sb[:], warm_sb[:, :64])
            nc.scalar.activation(warm_act[:], warm_ps[:], silu)
            for _ in range(56):
                nc.tensor.matmul(warm_ps[:], warm_sb[:], warm_sb[:, :64])

            for ci, (g0, w, e) in enumerate(chunks):
                if ci + 2 < len(chunks):
                    x_chunks.append(load_x_chunk(ci + 2))
                x_sb = x_chunks[ci]

                # phase 1: h_slice = silu(x @ W1_slice.T), F-major hs[f, tok]
                hs = hpool.tile([P, FTS, CHUNK], BF16, tag="hs", name="hs")[:, :, :w]
                for f in range(FTS):
                    ph = psh.tile([P, CHUNK], F32, tag="ph", name="ph")[:, :w]
                    for h in range(HT):
                        nc.tensor.matmul(
                            ph[:],
                            w1_sb[e][:, h, f * P : (f + 1) * P],
                            x_sb[:, h, :],
                            start=(h == 0),
                            stop=(h == HT - 1),
                        )
                    nc.scalar.activation(hs[:, f, :], ph[:], silu)

                for ek in w_sched.get(ci, ()):
                    load_w_expert(ek)

                # gemm 2, H-major: py[h-tile, tok] += w2[f, h-tile].T @ hs[f].
                # Two half-H passes keep PSUM at 4 banks.
                y_sb = ypool.tile([P, HT, CHUNK], BF16, tag="y", name="y")[:, :, :w]
                for hg in range(2):
                    # f-outer over the group's four h-tiles, deferring each
                    # group's f=3 round: the last silu (f=3) then has ~12
                    # matmul slots of cover instead of 3, and the psum->sbuf
                    # copies spread across the f=3 round.
                    h0 = hg * (HT // 2)
                    pyh = [
                        psy.tile([P, CHUNK], F32, tag=f"pyh_{hh}", name=f"pyh_{hh}")[:, :w]
                        for hh in range(HT // 2)
                    ]
                    for f in range(FTS - 1):
                        for hh in range(HT // 2):
                            nc.tensor.matmul(
                                pyh[hh][:],
                                w2_sb[e][:, f, (h0 + hh) * P : (h0 + hh + 1) * P],
                                hs[:, f, :],
                                start=(f == 0),
                                stop=False,
                            )
                    for hh in range(HT // 2):
                        nc.tensor.matmul(
                            pyh[hh][:],
                            w2_sb[e][:, FTS - 1, (h0 + hh) * P : (h0 + hh + 1) * P],
                            hs[:, FTS - 1, :],
                            start=False,
                            stop=True,
                        )
                        nc.vector.tensor_scalar_mul(
                            y_sb[:, h0 + hh, :], pyh[hh][:], 1.0
                        )
                    # store each half-H group as soon as its copies land, so
                    # the kernel tail only waits on the last half (fanned
                    # over four rings on the final chunk to shorten it).
                    ysplit = 4 if (ci == len(chunks) - 1 and hg == 1) else 1
                    hstep = (HT // 2) // ysplit
                    for s in range(ysplit):
                        hb = h0 + s * hstep
                        nc.sync.dma_start(
                            y_d[:, hb : hb + hstep, g0 : g0 + w],
                            y_sb[:, hb : hb + hstep, :],
                        )

    nc.compile()
    return nc


def _get_program(plan_key):
    if plan_key not in _program_cache:
        _program_cache[plan_key] = _build_program(plan_key)
    return _program_cache[plan_key]


def _route(topk_e, topk_w):
    """Per-expert token indices and combined gate weights (duplicate top-k
    hits of the same expert are merged by summing their weights, matching
    the reference's repeated +=)."""
    idxs, wts = [], []
    for e in range(E):
        m = topk_e == e
        idx = np.nonzero(m.any(axis=1))[0]
        we = (topk_w.astype(np.float32) * m).sum(axis=1)[idx]
        idxs.append(idx)
        wts.append(we)
    return idxs, wts


def _ensure_device_healthy():
    """Probe the accelerator; if wedged (NRT unrecoverable), axon_reset it.
    Best-effort: silently skips when not running under the axon proxy."""
    try:
        import jax
        import jax.numpy as jnp
    except Exception:
        return
    for _ in range(3):
        try:
            a = jnp.ones((8, 8))
            float((a @ a).sum())
            return
        except Exception:
            try:
                import ctypes

                lib = ctypes.CDLL("/opt/axon/libaxon_pjrt.so")
                lib.axon_reset.restype = ctypes.c_int64
                lib.axon_reset()
            except Exception:
                return


def kernel(x, topk_e, topk_w, w1, w2):
    global LAST_RESULTS
    _ensure_device_healthy()
    x = np.ascontiguousarray(np.asarray(x), dtype=np.float32)
    topk_e = np.asarray(topk_e)
    topk_w = np.asarray(topk_w)
    w1 = np.asarray(w1, dtype=np.float32)
    w2 = np.asarray(w2, dtype=np.float32)
    S = x.shape[0]

    idxs, wts = _route(topk_e, topk_w)
    counts = [len(i) for i in idxs]
    T = sum(counts)
    chunks = _make_plan(counts)
    plan_key = (tuple(chunks), T)

    nc = _get_program(plan_key)

    bf = ml_dtypes.bfloat16

    # Shared x stream, p-major [P, HT, T]: xt[p, h, t] = x[pair[t], h*128+p].
    pair_idx = np.concatenate(idxs)
    xs = x[pair_idx].astype(bf)                              # [T, H]
    xt = np.ascontiguousarray(xs.T.reshape(HT, P, T).transpose(1, 0, 2))

    in_maps = []
    for j in range(N_CORES):
        f0 = j * FS
        # w1t[e, p, h, c] = w1[e, f0+c, h*128+p]
        w1t = np.ascontiguousarray(
            w1[:, f0 : f0 + FS, :].astype(bf)
            .reshape(E, FS, HT, P).transpose(0, 3, 2, 1)
        )
        # w2t[e, p, f, c] = w2[e, c, f0 + f*128 + p]
        w2t = np.ascontiguousarray(
            w2[:, :, f0 : f0 + FS].astype(bf)
            .reshape(E, H, FTS, P).transpose(0, 3, 2, 1)
        )
        in_maps.append({"xt": xt, "w1t": w1t, "w2t": w2t})

    res = run_bass_kernel_spmd(nc, in_maps, core_ids=list(range(N_CORES)))
    LAST_RESULTS = res

    # y_d is [P, HT, T] bf16 per core; sum cores, transpose to [T, H],
    # apply the combine weight, scatter-add per expert segment.
    ysum = np.zeros((P, HT, T), np.float32)
    for j in range(N_CORES):
        ysum += res.results[j]["y"].astype(np.float32)
    yt = ysum.transpose(2, 1, 0).reshape(T, H)
    wc_stream = np.concatenate(wts).astype(np.float32)
    yt *= wc_stream[:, None]

    y = np.zeros((S, H), np.float32)
    g0 = 0
    for e in range(E):
        L = counts[e]
        y[idxs[e]] += yt[g0 : g0 + L]
        g0 += L
    return y


# revision 36
# speedup vs baseline: 1.1601x; 1.1601x over previous
"""Trainium2 Bass kernel for nn_ExpertMLP (MoE routing).

Strategy (tensor-parallel over d_ff, host-side dispatch):
  - Every core processes ALL T token-expert pairs (the concatenation of
    each expert's distinct hit tokens, duplicate top-k hits merged with
    summed gate weight), but only a 512-wide slice of the FFN dim F:
    core j holds w1[e, j*512:(j+1)*512, :] and w2[e, :, j*512:(j+1)*512]
    for all 8 experts (16 MB bf16 -- same footprint as one full expert).
  - silu is elementwise, so the F-slice passes through it exactly:
        part_j = silu(x @ W1_j.T) @ W2_j.T          # [T, H]
        y(pair) = wc(pair) * sum_j part_j(pair)
    The host sums the 8 bf16 partials, applies wc, and scatter-adds per
    expert segment into the full [S, H] output.
  - Work per core is T * 2*2*H*(F/8) MACs -- perfectly balanced by
    construction (no capacity padding, no expert imbalance), ~6% less
    than the expert-parallel layout's max-expert padding, and with the
    H-major second GEMM there is no partial-tile waste at all:
    the PE stream is exactly 64*T cycles.

Device kernel layout (per core):
  The pair stream is cut into per-expert chunks of <=512 tokens, so every
  chunk uses a single expert's (resident) weight slice. Per chunk:
    phase 1:  for f in 4:  ph[128, w] += w1_sb[e][h, f-tile].T @ x[h]   (8 MMs)
              silu(ph) -> hs[:, f, :]   (ACT engine)
    gemm 2:   for h in 8:  pyh[128, w] += w2_sb[e][f, h-tile].T @ hs[f]  (4 MMs)
              copy pyh -> y_sb[:, h, :]  (DVE, f32->bf16)
    one [128, 8, w] bf16 DMA out per chunk (y stays H-major; the host
    transposes once after summing cores).
  All DMAs ride the sync-engine HWDGE queue in consumption order, one
  enqueue per chunk / per weight tensor (the host pre-lays p-major
  layouts so each load is a single 3D access pattern). Expert k's
  weights are enqueued three chunks before their first use. The scalar
  engine runs ONLY activations (its table is preloaded during the PE
  warm-up) so silu is never stuck behind DMA enqueues.

  SBUF/partition: w1 64KB + w2 64KB + x 32KB + hs 8KB + y 16KB ~= 184KB.
  PSUM banks: 2 (ph) + 4 (pyh) + 2 (warm) = 8.
"""

import numpy as np
import ml_dtypes

import concourse.bacc as bacc
import concourse.mybir as mybir
import concourse.tile as tile
from concourse.bass_utils import run_bass_kernel_spmd

P = 128
H = 1024
F = 4096
E = 8
N_CORES = 8
CHUNK = 512
FS = F // N_CORES   # 512: per-core f-slice
FTS = FS // P       # 4 f-tiles per core
HT = H // P         # 8

BF16 = mybir.dt.bfloat16
F32 = mybir.dt.float32

# Results of the most recent device run (BassKernelResults); lets a test
# harness read exec_time_ns / trace paths without changing kernel()'s API.
LAST_RESULTS = None

_program_cache = {}


def _make_plan(counts):
    """Cut the concatenated per-expert pair stream into single-expert
    chunks of <=512 tokens. Returns a list of (g0, w, e)."""
    chunks = []
    g0 = 0
    for e, L in enumerate(counts):
        if L == 0:
            continue
        n = -(-L // CHUNK)
        widths = [CHUNK] * (n - 1) + [L - CHUNK * (n - 1)]
        if n >= 2 and widths[-1] < P:
            widths[-2] = CHUNK // 2
            widths[-1] += CHUNK // 2
        for w in widths:
            chunks.append((g0, w, e))
            g0 += w
    # Halve the first chunk: chunk 0's x bytes gate the first matmul, and
    # a small chunk 0 keeps the head DMA-feasible even when the chip is in
    # its throttled clock state.
    if chunks and chunks[0][1] == CHUNK:
        g0, w, e = chunks[0]
        chunks[0:1] = [(g0, w // 2, e), (g0 + w // 2, w - w // 2, e)]
    return chunks


def _build_program(plan_key):
    chunks, T = plan_key
    nc = bacc.Bacc(None, name="expert_mlp_tp")

    xt_d = nc.dram_tensor("xt", (P, HT, T), BF16, kind="ExternalInput")
    w1t_d = nc.dram_tensor("w1t", (E, P, HT, FS), BF16, kind="ExternalInput")
    w2t_d = nc.dram_tensor("w2t", (E, P, FTS, H), BF16, kind="ExternalInput")
    y_d = nc.dram_tensor("y", (P, HT, T), BF16, kind="ExternalOutput")

    silu = mybir.ActivationFunctionType.Silu

    # Emit expert k's weight load 3 chunks ahead of its first chunk.
    first_chunk = {}
    for ci, (_, _, e) in enumerate(chunks):
        first_chunk.setdefault(e, ci)
    w_sched = {}
    head_experts = []
    for e, fc in first_chunk.items():
        at = fc - 3
        if at <= 0:
            head_experts.append(e)
        else:
            w_sched.setdefault(at, []).append(e)

    with tile.TileContext(nc) as tc:
        with (
            tc.tile_pool(name="wpool", bufs=1) as wpool,
            tc.tile_pool(name="xpool", bufs=4) as xpool,
            tc.tile_pool(name="hpool", bufs=2) as hpool,
            tc.tile_pool(name="ypool", bufs=2) as ypool,
            tc.tile_pool(name="spool", bufs=1) as spool,
            tc.tile_pool(name="psh", bufs=2, space="PSUM") as psh,
            tc.tile_pool(name="psy", bufs=1, space="PSUM") as psy,
        ):
            w1_sb = [
                wpool.tile([P, HT, FS], BF16, tag=f"w1_{e}", name=f"w1_{e}")
                for e in range(E)
            ]
            w2_sb = [
                wpool.tile([P, FTS, H], BF16, tag=f"w2_{e}", name=f"w2_{e}")
                for e in range(E)
            ]

            def load_w_expert(e):
                # Steady-state expert loads ride the scalar HWDGE queue: the
                # sync engine's per-chunk enqueue budget (x + y) is nearly
                # full, and a weight burst there can push an x prefetch past
                # its deadline (observed as a 12us stall). These are emitted
                # AFTER a chunk's four ACTs so silu never queues behind them.
                nc.scalar.dma_start(w1_sb[e][:], w1t_d[e])
                nc.scalar.dma_start(w2_sb[e][:], w2t_d[e])

            def load_x_chunk(ci, split=1):
                # split>1 fans the load over several DMA rings -- used for
                # chunk 0, whose arrival gates the first real matmul.
                g0, w, _ = chunks[ci]
                t = xpool.tile([P, HT, CHUNK], BF16, tag="x", name="x")[:, :, :w]
                hstep = HT // split
                for s in range(split):
                    h0 = s * hstep
                    nc.sync.dma_start(
                        t[:, h0 : h0 + hstep, :],
                        xt_d[:, h0 : h0 + hstep, g0 : g0 + w],
                    )
                return t

            # Head DMAs, all on the sync queue (scalar-queue enqueues proved
            # slow and block the ACTs behind them). Enqueue instructions
            # cost ~0.6us of engine time each, so the order interleaves the
            # two tensors the first matmul actually waits on: the first
            # half of x chunk 0 and w1's first f-tile quarter. w2 is only
            # needed once gemm 2 starts and goes last.
            g0_0, w_0, _ = chunks[0]
            x_t0 = xpool.tile([P, HT, CHUNK], BF16, tag="x", name="x")[:, :, :w_0]
            hh = HT // 2
            nc.sync.dma_start(x_t0[:, :hh, :], xt_d[:, :hh, g0_0 : g0_0 + w_0])
            e0 = sorted(head_experts)[0] if head_experts else None
            if e0 is not None:
                nc.sync.dma_start(
                    w1_sb[e0][:, :, 0:P], w1t_d[e0][:, :, 0:P]
                )
            nc.sync.dma_start(x_t0[:, hh:, :], xt_d[:, hh:, g0_0 : g0_0 + w_0])
            if e0 is not None:
                for f in range(1, FTS):
                    nc.sync.dma_start(
                        w1_sb[e0][:, :, f * P : (f + 1) * P],
                        w1t_d[e0][:, :, f * P : (f + 1) * P],
                    )
            x_chunks = [x_t0]
            if len(chunks) > 1:
                x_chunks.append(load_x_chunk(1))
            for e in sorted(head_experts):
                if e != e0:
                    nc.sync.dma_start(w1_sb[e][:], w1t_d[e])
                # split by H half: gemm 2's first half-group only needs
                # columns 0:512 (all f), so it can start off the first DMA.
                nc.sync.dma_start(w2_sb[e][:, :, 0 : H // 2], w2t_d[e][:, :, 0 : H // 2])
                nc.sync.dma_start(w2_sb[e][:, :, H // 2 :], w2t_d[e][:, :, H // 2 :])

            # PE pre-warm: dependency-free matmuls on a zeroed scratch tile
            # run while the head DMAs are in flight, so the HAM clock gate
            # is at 8/8 when the real matmuls start. One of them feeds a
            # throwaway silu so the ACT table loads off the critical path.
            warm_sb = spool.tile([P, P], BF16, tag="warm", name="warm_sb")
            warm_act = spool.tile([P, 64], BF16, tag="warm_act", name="warm_act")
            nc.gpsimd.memset(warm_sb[:], 0.0)
            warm_ps = psy.tile([P, 64], F32, tag="warm_ps", name="warm_ps")
            warm_ps2 = psy.tile([P, 64], F32, tag="warm_ps2", name="warm_ps2")
            nc.tensor.matmul(warm_ps2[:], warm_sb[:], warm_sb[:, :64])
            nc.scalar.activation(warm_act[:], warm_ps2[:], silu)
            for _ in range(56):
                nc.tensor.matmul(warm_ps[:], warm_sb[:], warm_sb[:, :64])

            for ci, (g0, w, e) in enumerate(chunks):
                if ci + 2 < len(chunks):
                    x_chunks.append(load_x_chunk(ci + 2))
                x_sb = x_chunks[ci]

                # phase 1: h_slice = silu(x @ W1_slice.T), F-major hs[f, tok]
                hs = hpool.tile([P, FTS, CHUNK], BF16, tag="hs", name="hs")[:, :, :w]
                for f in range(FTS):
                    ph = psh.tile([P, CHUNK], F32, tag="ph", name="ph")[:, :w]
                    for h in range(HT):
                        nc.tensor.matmul(
                            ph[:],
                            w1_sb[e][:, h, f * P : (f + 1) * P],
                            x_sb[:, h, :],
                            start=(h == 0),
                            stop=(h == HT - 1),
                        )
                    nc.scalar.activation(hs[:, f, :], ph[:], silu)

                for ek in w_sched.get(ci, ()):
                    load_w_expert(ek)

                # gemm 2, H-major: py[h-tile, tok] += w2[f, h-tile].T @ hs[f].
                # Two half-H passes keep PSUM at 4 banks.
                y_sb = ypool.tile([P, HT, CHUNK], BF16, tag="y", name="y")[:, :, :w]
                for hg in range(2):
                    # f-outer over the group's four h-tiles, deferring each
                    # group's f=3 round: the last silu (f=3) then has ~12
                    # matmul slots of cover instead of 3, and the psum->sbuf
                    # copies spread across the f=3 round.
                    h0 = hg * (HT // 2)
                    pyh = [
                        psy.tile([P, CHUNK], F32, tag=f"pyh_{hh}", name=f"pyh_{hh}")[:, :w]
                        for hh in range(HT // 2)
                    ]
                    for f in range(FTS - 1):
                        for hh in range(HT // 2):
                            nc.tensor.matmul(
                                pyh[hh][:],
                                w2_sb[e][:, f, (h0 + hh) * P : (h0 + hh + 1) * P],
                                hs[:, f, :],
                                start=(f == 0),
                                stop=False,
                            )
                    for hh in range(HT // 2):
                        nc.tensor.matmul(
                            pyh[hh][:],
                            w2_sb[e][:, FTS - 1, (h0 + hh) * P : (h0 + hh + 1) * P],
                            hs[:, FTS - 1, :],
                            start=False,
                            stop=True,
                        )
                        nc.vector.tensor_scalar_mul(
                            y_sb[:, h0 + hh, :], pyh[hh][:], 1.0
                        )
                    # store each half-H group as soon as its copies land, so
                    # the kernel tail only waits on the last half (fanned
                    # over two rings on the final chunk to shorten it).
                    ysplit = 2 if (ci == len(chunks) - 1 and hg == 1) else 1
                    hstep = (HT // 2) // ysplit
                    for s in range(ysplit):
                        hb = h0 + s * hstep
                        nc.sync.dma_start(
                            y_d[:, hb : hb + hstep, g0 : g0 + w],
                            y_sb[:, hb : hb + hstep, :],
                        )

    nc.compile()
    return nc


def _get_program(plan_key):
    if plan_key not in _program_cache:
        _program_cache[plan_key] = _build_program(plan_key)
    return _program_cache[plan_key]


def _route(topk_e, topk_w):
    """Per-expert token indices and combined gate weights (duplicate top-k
    hits of the same expert are merged by summing their weights, matching
    the reference's repeated +=)."""
    idxs, wts = [], []
    for e in range(E):
        m = topk_e == e
        idx = np.nonzero(m.any(axis=1))[0]
        we = (topk_w.astype(np.float32) * m).sum(axis=1)[idx]
        idxs.append(idx)
        wts.append(we)
    return idxs, wts


def _ensure_device_healthy():
    """Probe the accelerator; if wedged (NRT unrecoverable), axon_reset it.
    Best-effort: silently skips when not running under the axon proxy."""
    try:
        import jax
        import jax.numpy as jnp
    except Exception:
        return
    for _ in range(3):
        try:
            a = jnp.ones((8, 8))
            float((a @ a).sum())
            return
        except Exception:
            try:
                import ctypes

                lib = ctypes.CDLL("/opt/axon/libaxon_pjrt.so")
                lib.axon_reset.restype = ctypes.c_int64
                lib.axon_reset()
            except Exception:
                return


def kernel(x, topk_e, topk_w, w1, w2):
    global LAST_RESULTS
    _ensure_device_healthy()
    x = np.ascontiguousarray(np.asarray(x), dtype=np.float32)
    topk_e = np.asarray(topk_e)
    topk_w = np.asarray(topk_w)
    w1 = np.asarray(w1, dtype=np.float32)
    w2 = np.asarray(w2, dtype=np.float32)
    S = x.shape[0]

    idxs, wts = _route(topk_e, topk_w)
    counts = [len(i) for i in idxs]
    T = sum(counts)
    chunks = _make_plan(counts)
    plan_key = (tuple(chunks), T)

    nc = _get_program(plan_key)

    bf = ml_dtypes.bfloat16

    # Shared x stream, p-major [P, HT, T]: xt[p, h, t] = x[pair[t], h*128+p].
    pair_idx = np.concatenate(idxs)
    xs = x[pair_idx].astype(bf)                              # [T, H]
    xt = np.ascontiguousarray(xs.T.reshape(HT, P, T).transpose(1, 0, 2))

    in_maps = []
    for j in range(N_CORES):
        f0 = j * FS
        # w1t[e, p, h, c] = w1[e, f0+c, h*128+p]
        w1t = np.ascontiguousarray(
            w1[:, f0 : f0 + FS, :].astype(bf)
            .reshape(E, FS, HT, P).transpose(0, 3, 2, 1)
        )
        # w2t[e, p, f, c] = w2[e, c, f0 + f*128 + p]
        w2t = np.ascontiguousarray(
            w2[:, :, f0 : f0 + FS].astype(bf)
            .reshape(E, H, FTS, P).transpose(0, 3, 2, 1)
        )
        in_maps.append({"xt": xt, "w1t": w1t, "w2t": w2t})

    res = run_bass_kernel_spmd(nc, in_maps, core_ids=list(range(N_CORES)))
    LAST_RESULTS = res

    # y_d is [P, HT, T] bf16 per core; sum cores, transpose to [T, H],
    # apply the combine weight, scatter-add per expert segment.
    ysum = np.zeros((P, HT, T), np.float32)
    for j in range(N_CORES):
        ysum += res.results[j]["y"].astype(np.float32)
    yt = ysum.transpose(2, 1, 0).reshape(T, H)
    wc_stream = np.concatenate(wts).astype(np.float32)
    yt *= wc_stream[:, None]

    y = np.zeros((S, H), np.float32)
    g0 = 0
    for e in range(E):
        L = counts[e]
        y[idxs[e]] += yt[g0 : g0 + L]
        g0 += L
    return y


# revision 40
# speedup vs baseline: 1.2225x; 1.0537x over previous
"""Trainium2 Bass kernel for nn_ExpertMLP (MoE routing).

Strategy (tensor-parallel over d_ff, host-side dispatch):
  - Every core processes ALL T token-expert pairs (the concatenation of
    each expert's distinct hit tokens, duplicate top-k hits merged with
    summed gate weight), but only a 512-wide slice of the FFN dim F:
    core j holds w1[e, j*512:(j+1)*512, :] and w2[e, :, j*512:(j+1)*512]
    for all 8 experts (16 MB bf16 -- same footprint as one full expert).
  - silu is elementwise, so the F-slice passes through it exactly:
        part_j = silu(x @ W1_j.T) @ W2_j.T          # [T, H]
        y(pair) = wc(pair) * sum_j part_j(pair)
    The host sums the 8 bf16 partials, applies wc, and scatter-adds per
    expert segment into the full [S, H] output.
  - Work per core is T * 2*2*H*(F/8) MACs -- perfectly balanced by
    construction (no capacity padding, no expert imbalance), ~6% less
    than the expert-parallel layout's max-expert padding, and with the
    H-major second GEMM there is no partial-tile waste at all:
    the PE stream is exactly 64*T cycles.

Device kernel layout (per core):
  The pair stream is cut into per-expert chunks of <=512 tokens, so every
  chunk uses a single expert's (resident) weight slice. Per chunk:
    phase 1:  for f in 4:  ph[128, w] += w1_sb[e][h, f-tile].T @ x[h]   (8 MMs)
              silu(ph) -> hs[:, f, :]   (ACT engine)
    gemm 2:   for h in 8:  pyh[128, w] += w2_sb[e][f, h-tile].T @ hs[f]  (4 MMs)
              copy pyh -> y_sb[:, h, :]  (DVE, f32->bf16)
    one [128, 8, w] bf16 DMA out per chunk (y stays H-major; the host
    transposes once after summing cores).
  All DMAs ride the sync-engine HWDGE queue in consumption order, one
  enqueue per chunk / per weight tensor (the host pre-lays p-major
  layouts so each load is a single 3D access pattern). Expert k's
  weights are enqueued three chunks before their first use. The scalar
  engine runs ONLY activations (its table is preloaded during the PE
  warm-up) so silu is never stuck behind DMA enqueues.

  SBUF/partition: w1 64KB + w2 64KB + x 32KB + hs 8KB + y 16KB ~= 184KB.
  PSUM banks: 2 (ph) + 4 (pyh) + 2 (warm) = 8.
"""

import numpy as np
import ml_dtypes

import concourse.bacc as bacc
import concourse.mybir as mybir
import concourse.tile as tile
from concourse.bass_utils import run_bass_kernel_spmd

P = 128
H = 1024
F = 4096
E = 8
N_CORES = 8
CHUNK = 512
FS = F // N_CORES   # 512: per-core f-slice
FTS = FS // P       # 4 f-tiles per core
HT = H // P         # 8

BF16 = mybir.dt.bfloat16
F32 = mybir.dt.float32

# Results of the most recent device run (BassKernelResults); lets a test
# harness read exec_time_ns / trace paths without changing kernel()'s API.
LAST_RESULTS = None

_program_cache = {}


def _make_plan(counts):
    """Cut the concatenated per-expert pair stream into single-expert
    chunks of <=512 tokens. Returns a list of (g0, w, e)."""
    chunks = []
    g0 = 0
    for e, L in enumerate(counts):
        if L == 0:
            continue
        n = -(-L // CHUNK)
        widths = [CHUNK] * (n - 1) + [L - CHUNK * (n - 1)]
        if n >= 2 and widths[-1] < P:
            widths[-2] = CHUNK // 2
            widths[-1] += CHUNK // 2
        for w in widths:
            chunks.append((g0, w, e))
            g0 += w
    return chunks


def _build_program(plan_key):
    chunks, T = plan_key
    nc = bacc.Bacc(None, name="expert_mlp_tp")

    xt_d = nc.dram_tensor("xt", (P, HT, T), BF16, kind="ExternalInput")
    w1t_d = nc.dram_tensor("w1t", (E, P, HT, FS), BF16, kind="ExternalInput")
    w2t_d = nc.dram_tensor("w2t", (E, P, FTS, H), BF16, kind="ExternalInput")
    y_d = nc.dram_tensor("y", (P, HT, T), BF16, kind="ExternalOutput")

    silu = mybir.ActivationFunctionType.Silu

    # Emit expert k's weight load 3 chunks ahead of its first chunk.
    first_chunk = {}
    for ci, (_, _, e) in enumerate(chunks):
        first_chunk.setdefault(e, ci)
    w_sched = {}
    head_experts = []
    for e, fc in first_chunk.items():
        at = fc - 3
        if at <= 0:
            head_experts.append(e)
        else:
            w_sched.setdefault(at, []).append(e)

    with tile.TileContext(nc) as tc:
        with (
            tc.tile_pool(name="wpool", bufs=1) as wpool,
            tc.tile_pool(name="xpool", bufs=4) as xpool,
            tc.tile_pool(name="hpool", bufs=2) as hpool,
            tc.tile_pool(name="ypool", bufs=2) as ypool,
            tc.tile_pool(name="spool", bufs=1) as spool,
            tc.tile_pool(name="psh", bufs=2, space="PSUM") as psh,
            tc.tile_pool(name="psy", bufs=1, space="PSUM") as psy,
        ):
            w1_sb = [
                wpool.tile([P, HT, FS], BF16, tag=f"w1_{e}", name=f"w1_{e}")
                for e in range(E)
            ]
            w2_sb = [
                wpool.tile([P, FTS, H], BF16, tag=f"w2_{e}", name=f"w2_{e}")
                for e in range(E)
            ]

            def load_w_expert(e):
                nc.sync.dma_start(w1_sb[e][:], w1t_d[e])
                nc.sync.dma_start(w2_sb[e][:], w2t_d[e])

            def load_x_chunk(ci, split=1):
                # split>1 fans the load over several DMA rings -- used for
                # chunk 0, whose arrival gates the first real matmul.
                g0, w, _ = chunks[ci]
                t = xpool.tile([P, HT, CHUNK], BF16, tag="x", name="x")[:, :, :w]
                hstep = HT // split
                for s in range(split):
                    h0 = s * hstep
                    nc.sync.dma_start(
                        t[:, h0 : h0 + hstep, :],
                        xt_d[:, h0 : h0 + hstep, g0 : g0 + w],
                    )
                return t

            # Head DMAs, all on the sync queue (scalar-queue enqueues proved
            # slow and block the ACTs behind them). Enqueue instructions
            # cost ~0.6us of engine time each, so the order interleaves the
            # two tensors the first matmul actually waits on: the first
            # half of x chunk 0 and w1's first f-tile quarter. w2 is only
            # needed once gemm 2 starts and goes last.
            g0_0, w_0, _ = chunks[0]
            x_t0 = xpool.tile([P, HT, CHUNK], BF16, tag="x", name="x")[:, :, :w_0]
            hh = HT // 2
            nc.sync.dma_start(x_t0[:, :hh, :], xt_d[:, :hh, g0_0 : g0_0 + w_0])
            e0 = sorted(head_experts)[0] if head_experts else None
            if e0 is not None:
                nc.sync.dma_start(
                    w1_sb[e0][:, :, 0:P], w1t_d[e0][:, :, 0:P]
                )
            nc.sync.dma_start(x_t0[:, hh:, :], xt_d[:, hh:, g0_0 : g0_0 + w_0])
            if e0 is not None:
                for f in range(1, FTS):
                    nc.sync.dma_start(
                        w1_sb[e0][:, :, f * P : (f + 1) * P],
                        w1t_d[e0][:, :, f * P : (f + 1) * P],
                    )
            x_chunks = [x_t0]
            if len(chunks) > 1:
                x_chunks.append(load_x_chunk(1))
            for e in sorted(head_experts):
                if e != e0:
                    nc.sync.dma_start(w1_sb[e][:], w1t_d[e])
                # split by H half: gemm 2's first half-group only needs
                # columns 0:512 (all f), so it can start off the first DMA.
                nc.sync.dma_start(w2_sb[e][:, :, 0 : H // 2], w2t_d[e][:, :, 0 : H // 2])
                nc.sync.dma_start(w2_sb[e][:, :, H // 2 :], w2t_d[e][:, :, H // 2 :])

            # PE pre-warm: dependency-free matmuls on a zeroed scratch tile
            # run while the head DMAs are in flight, so the HAM clock gate
            # is at 8/8 when the real matmuls start. One of them feeds a
            # throwaway silu so the ACT table loads off the critical path.
            warm_sb = spool.tile([P, P], BF16, tag="warm", name="warm_sb")
            warm_act = spool.tile([P, 64], BF16, tag="warm_act", name="warm_act")
            nc.gpsimd.memset(warm_sb[:], 0.0)
            warm_ps = psy.tile([P, 64], F32, tag="warm_ps", name="warm_ps")
            warm_ps2 = psy.tile([P, 64], F32, tag="warm_ps2", name="warm_ps2")
            nc.tensor.matmul(warm_ps2[:], warm_sb[:], warm_sb[:, :64])
            nc.scalar.activation(warm_act[:], warm_ps2[:], silu)
            for _ in range(56):
                nc.tensor.matmul(warm_ps[:], warm_sb[:], warm_sb[:, :64])

            for ci, (g0, w, e) in enumerate(chunks):
                if ci + 2 < len(chunks):
                    x_chunks.append(load_x_chunk(ci + 2))
                for ek in w_sched.get(ci, ()):
                    load_w_expert(ek)
                x_sb = x_chunks[ci]

                # phase 1: h_slice = silu(x @ W1_slice.T), F-major hs[f, tok]
                hs = hpool.tile([P, FTS, CHUNK], BF16, tag="hs", name="hs")[:, :, :w]
                for f in range(FTS):
                    ph = psh.tile([P, CHUNK], F32, tag="ph", name="ph")[:, :w]
                    for h in range(HT):
                        nc.tensor.matmul(
                            ph[:],
                            w1_sb[e][:, h, f * P : (f + 1) * P],
                            x_sb[:, h, :],
                            start=(h == 0),
                            stop=(h == HT - 1),
                        )
                    nc.scalar.activation(hs[:, f, :], ph[:], silu)

                # gemm 2, H-major: py[h-tile, tok] += w2[f, h-tile].T @ hs[f].
                # Two half-H passes keep PSUM at 4 banks.
                y_sb = ypool.tile([P, HT, CHUNK], BF16, tag="y", name="y")[:, :, :w]
                for hg in range(2):
                    # f-outer over the group's four h-tiles, deferring each
                    # group's f=3 round: the last silu (f=3) then has ~12
                    # matmul slots of cover instead of 3, and the psum->sbuf
                    # copies spread across the f=3 round.
                    h0 = hg * (HT // 2)
                    pyh = [
                        psy.tile([P, CHUNK], F32, tag=f"pyh_{hh}", name=f"pyh_{hh}")[:, :w]
                        for hh in range(HT // 2)
                    ]
                    for f in range(FTS - 1):
                        for hh in range(HT // 2):
                            nc.tensor.matmul(
                                pyh[hh][:],
                                w2_sb[e][:, f, (h0 + hh) * P : (h0 + hh + 1) * P],
                                hs[:, f, :],
                                start=(f == 0),
                                stop=False,
                            )
                    for hh in range(HT // 2):
                        nc.tensor.matmul(
                            pyh[hh][:],
                            w2_sb[e][:, FTS - 1, (h0 + hh) * P : (h0 + hh + 1) * P],
                            hs[:, FTS - 1, :],
                            start=False,
                            stop=True,
                        )
                        nc.vector.tensor_scalar_mul(
                            y_sb[:, h0 + hh, :], pyh[hh][:], 1.0
                        )
                    # store each half-H group as soon as its copies land, so
                    # the kernel tail only waits on the last half (fanned
                    # over two rings on the final chunk to shorten it).
                    ysplit = 2 if (ci == len(chunks) - 1 and hg == 1) else 1
                    hstep = (HT // 2) // ysplit
                    for s in range(ysplit):
                        hb = h0 + s * hstep
                        nc.sync.dma_start(
                            y_d[:, hb : hb + hstep, g0 : g0 + w],
                            y_sb[:, hb : hb + hstep, :],
                        )

    nc.compile()
    return nc


def _get_program(plan_key):
    if plan_key not in _program_cache:
        _program_cache[plan_key] = _build_program(plan_key)
    return _program_cache[plan_key]


def _route(topk_e, topk_w):
    """Per-expert token indices and combined gate weights (duplicate top-k
    hits of the same expert are merged by summing their weights, matching
    the reference's repeated +=)."""
    idxs, wts = [], []
    for e in range(E):
        m = topk_e == e
        idx = np.nonzero(m.any(axis=1))[0]
        we = (topk_w.astype(np.float32) * m).sum(axis=1)[idx]
        idxs.append(idx)
        wts.append(we)
    return idxs, wts


def _ensure_device_healthy():
    """Probe the accelerator; if wedged (NRT unrecoverable), axon_reset it.
    Best-effort: silently skips when not running under the axon proxy."""
    try:
        import jax
        import jax.numpy as jnp
    except Exception:
        return
    for _ in range(3):
        try:
            a = jnp.ones((8, 8))
            float((a @ a).sum())
            return
        except Exception:
            try:
                import ctypes

                lib = ctypes.CDLL("/opt/axon/libaxon_pjrt.so")
                lib.axon_reset.restype = ctypes.c_int64
                lib.axon_reset()
            except Exception:
                return


def kernel(x, topk_e, topk_w, w1, w2):
    global LAST_RESULTS
    _ensure_device_healthy()
    x = np.ascontiguousarray(np.asarray(x), dtype=np.float32)
    topk_e = np.asarray(topk_e)
    topk_w = np.asarray(topk_w)
    w1 = np.asarray(w1, dtype=np.float32)
    w2 = np.asarray(w2, dtype=np.float32)
    S = x.shape[0]

    idxs, wts = _route(topk_e, topk_w)
    counts = [len(i) for i in idxs]
    T = sum(counts)
    chunks = _make_plan(counts)
    plan_key = (tuple(chunks), T)

    nc = _get_program(plan_key)

    bf = ml_dtypes.bfloat16

    # Shared x stream, p-major [P, HT, T]: xt[p, h, t] = x[pair[t], h*128+p].
    pair_idx = np.concatenate(idxs)
    xs = x[pair_idx].astype(bf)                              # [T, H]
    xt = np.ascontiguousarray(xs.T.reshape(HT, P, T).transpose(1, 0, 2))

    in_maps = []
    for j in range(N_CORES):
        f0 = j * FS
        # w1t[e, p, h, c] = w1[e, f0+c, h*128+p]
        w1t = np.ascontiguousarray(
            w1[:, f0 : f0 + FS, :].astype(bf)
            .reshape(E, FS, HT, P).transpose(0, 3, 2, 1)
        )
        # w2t[e, p, f, c] = w2[e, c, f0 + f*128 + p]
        w2t = np.ascontiguousarray(
            w2[:, :, f0 : f0 + FS].astype(bf)
            .reshape(E, H, FTS, P).transpose(0, 3, 2, 1)
        )
        in_maps.append({"xt": xt, "w1t": w1t, "w2t": w2t})

    res = run_bass_kernel_spmd(nc, in_maps, core_ids=list(range(N_CORES)))
    LAST_RESULTS = res

    # y_d is [P, HT, T] bf16 per core; sum cores, transpose to [T, H],
    # apply the combine weight, scatter-add per expert segment.
    ysum = np.zeros((P, HT, T), np.float32)
    for j in range(N_CORES):
        ysum += res.results[j]["y"].astype(np.float32)
    yt = ysum.transpose(2, 1, 0).reshape(T, H)
    wc_stream = np.concatenate(wts).astype(np.float32)
    yt *= wc_stream[:, None]

    y = np.zeros((S, H), np.float32)
    g0 = 0
    for e in range(E):
        L = counts[e]
        y[idxs[e]] += yt[g0 : g0 + L]
        g0 += L
    return y


# revision 47
# speedup vs baseline: 1.2314x; 1.0073x over previous
"""Trainium2 Bass kernel for nn_ExpertMLP (MoE routing).

Strategy (tensor-parallel over d_ff, host-side dispatch):
  - Every core processes ALL T token-expert pairs (the concatenation of
    each expert's distinct hit tokens, duplicate top-k hits merged with
    summed gate weight), but only a 512-wide slice of the FFN dim F:
    core j holds w1[e, j*512:(j+1)*512, :] and w2[e, :, j*512:(j+1)*512]
    for all 8 experts (16 MB bf16 -- same footprint as one full expert).
  - silu is elementwise, so the F-slice passes through it exactly:
        part_j = silu(x @ W1_j.T) @ W2_j.T          # [T, H]
        y(pair) = wc(pair) * sum_j part_j(pair)
    The host sums the 8 bf16 partials, applies wc, and scatter-adds per
    expert segment into the full [S, H] output.
  - Work per core is T * 2*2*H*(F/8) MACs -- perfectly balanced by
    construction (no capacity padding, no expert imbalance), ~6% less
    than the expert-parallel layout's max-expert padding, and with the
    H-major second GEMM there is no partial-tile waste at all:
    the PE stream is exactly 64*T cycles.

Device kernel layout (per core):
  The pair stream is cut into per-expert chunks of <=512 tokens, so every
  chunk uses a single expert's (resident) weight slice. Per chunk:
    phase 1:  for f in 4:  ph[128, w] += w1_sb[e][h, f-tile].T @ x[h]   (8 MMs)
              silu(ph) -> hs[:, f, :]   (ACT engine)
    gemm 2:   for h in 8:  pyh[128, w] += w2_sb[e][f, h-tile].T @ hs[f]  (4 MMs)
              copy pyh -> y_sb[:, h, :]  (DVE, f32->bf16)
    one [128, 8, w] bf16 DMA out per chunk (y stays H-major; the host
    transposes once after summing cores).
  All DMAs ride the sync-engine HWDGE queue in consumption order, one
  enqueue per chunk / per weight tensor (the host pre-lays p-major
  layouts so each load is a single 3D access pattern). Expert k's
  weights are enqueued three chunks before their first use. The scalar
  engine runs ONLY activations (its table is preloaded during the PE
  warm-up) so silu is never stuck behind DMA enqueues.

  SBUF/partition: w1 64KB + w2 64KB + x 32KB + hs 8KB + y 16KB ~= 184KB.
  PSUM banks: 2 (ph) + 4 (pyh) + 2 (warm) = 8.
"""

import numpy as np
import ml_dtypes

import concourse.bacc as bacc
import concourse.mybir as mybir
import concourse.tile as tile
from concourse.bass_utils import run_bass_kernel_spmd

P = 128
H = 1024
F = 4096
E = 8
N_CORES = 8
CHUNK = 512
FS = F // N_CORES   # 512: per-core f-slice
FTS = FS // P       # 4 f-tiles per core
HT = H // P         # 8

BF16 = mybir.dt.bfloat16
F32 = mybir.dt.float32

# Results of the most recent device run (BassKernelResults); lets a test
# harness read exec_time_ns / trace paths without changing kernel()'s API.
LAST_RESULTS = None

_program_cache = {}


def _make_plan(counts):
    """Cut the concatenated per-expert pair stream into single-expert
    chunks of <=512 tokens. Returns a list of (g0, w, e)."""
    chunks = []
    g0 = 0
    for e, L in enumerate(counts):
        if L == 0:
            continue
        n = -(-L // CHUNK)
        widths = [CHUNK] * (n - 1) + [L - CHUNK * (n - 1)]
        if n >= 2 and widths[-1] < P:
            widths[-2] = CHUNK // 2
            widths[-1] += CHUNK // 2
        for w in widths:
            chunks.append((g0, w, e))
            g0 += w
    return chunks


def _build_program(plan_key):
    chunks, T = plan_key
    nc = bacc.Bacc(None, name="expert_mlp_tp")

    xt_d = nc.dram_tensor("xt", (P, HT, T), BF16, kind="ExternalInput")
    # w1 is quarter-major: one fully-contiguous [P, HT*P] block per
    # (expert, f-tile), so each w1 DMA is 128 lines of 2KB and phase 1's
    # f=0 only depends on the first quarter's transfer.
    w1t_d = nc.dram_tensor("w1t", (E, FTS, P, HT, P), BF16, kind="ExternalInput")
    w2t_d = nc.dram_tensor("w2t", (E, P, FTS, H), BF16, kind="ExternalInput")
    y_d = nc.dram_tensor("y", (P, HT, T), BF16, kind="ExternalOutput")

    silu = mybir.ActivationFunctionType.Silu

    # Emit expert k's weight load 3 chunks ahead of its first chunk.
    first_chunk = {}
    for ci, (_, _, e) in enumerate(chunks):
        first_chunk.setdefault(e, ci)
    w_sched = {}
    head_experts = []
    for e, fc in first_chunk.items():
        at = fc - 3
        if at <= 0:
            head_experts.append(e)
        else:
            w_sched.setdefault(at, []).append(e)

    with tile.TileContext(nc) as tc:
        with (
            tc.tile_pool(name="wpool", bufs=1) as wpool,
            tc.tile_pool(name="xpool", bufs=4) as xpool,
            tc.tile_pool(name="hpool", bufs=2) as hpool,
            tc.tile_pool(name="ypool", bufs=2) as ypool,
            tc.tile_pool(name="spool", bufs=1) as spool,
            tc.tile_pool(name="psh", bufs=2, space="PSUM") as psh,
            tc.tile_pool(name="psy", bufs=1, space="PSUM") as psy,
        ):
            w1_sb = [
                [
                    wpool.tile([P, HT, P], BF16, tag=f"w1_{e}_{f}", name=f"w1_{e}_{f}")
                    for f in range(FTS)
                ]
                for e in range(E)
            ]
            w2_sb = [
                wpool.tile([P, FTS, H], BF16, tag=f"w2_{e}", name=f"w2_{e}")
                for e in range(E)
            ]

            def load_w_expert(e):
                for f in range(FTS):
                    nc.sync.dma_start(w1_sb[e][f][:], w1t_d[e, f])
                nc.sync.dma_start(w2_sb[e][:], w2t_d[e])

            def load_x_chunk(ci, split=1):
                # split>1 fans the load over several DMA rings -- used for
                # chunk 0, whose arrival gates the first real matmul.
                g0, w, _ = chunks[ci]
                t = xpool.tile([P, HT, CHUNK], BF16, tag="x", name="x")[:, :, :w]
                hstep = HT // split
                for s in range(split):
                    h0 = s * hstep
                    nc.sync.dma_start(
                        t[:, h0 : h0 + hstep, :],
                        xt_d[:, h0 : h0 + hstep, g0 : g0 + w],
                    )
                return t

            # Head DMAs, all on the sync queue (scalar-queue enqueues proved
            # slow and block the ACTs behind them). Enqueue instructions
            # cost ~0.6us of engine time each, so the order interleaves the
            # two tensors the first matmul actually waits on: the first
            # half of x chunk 0 and w1's first f-tile quarter. w2 is only
            # needed once gemm 2 starts and goes last.
            g0_0, w_0, _ = chunks[0]
            x_t0 = xpool.tile([P, HT, CHUNK], BF16, tag="x", name="x")[:, :, :w_0]
            hh = HT // 2
            nc.sync.dma_start(x_t0[:, :hh, :], xt_d[:, :hh, g0_0 : g0_0 + w_0])
            e0 = sorted(head_experts)[0] if head_experts else None
            if e0 is not None:
                nc.sync.dma_start(w1_sb[e0][0][:], w1t_d[e0, 0])
            nc.sync.dma_start(x_t0[:, hh:, :], xt_d[:, hh:, g0_0 : g0_0 + w_0])
            if e0 is not None:
                for f in range(1, FTS):
                    nc.sync.dma_start(w1_sb[e0][f][:], w1t_d[e0, f])
            x_chunks = [x_t0]
            if len(chunks) > 1:
                x_chunks.append(load_x_chunk(1))
            for e in sorted(head_experts):
                if e != e0:
                    for f in range(FTS):
                        nc.sync.dma_start(w1_sb[e][f][:], w1t_d[e, f])
                # split by H half: gemm 2's first half-group only needs
                # columns 0:512 (all f), so it can start off the first DMA.
                nc.sync.dma_start(w2_sb[e][:, :, 0 : H // 2], w2t_d[e][:, :, 0 : H // 2])
                nc.sync.dma_start(w2_sb[e][:, :, H // 2 :], w2t_d[e][:, :, H // 2 :])

            # PE pre-warm: dependency-free matmuls on a zeroed scratch tile
            # run while the head DMAs are in flight, so the HAM clock gate
            # is at 8/8 when the real matmuls start. One of them feeds a
            # throwaway silu so the ACT table loads off the critical path.
            warm_sb = spool.tile([P, P], BF16, tag="warm", name="warm_sb")
            warm_act = spool.tile([P, 64], BF16, tag="warm_act", name="warm_act")
            nc.gpsimd.memset(warm_sb[:], 0.0)
            warm_ps = psy.tile([P, 64], F32, tag="warm_ps", name="warm_ps")
            warm_ps2 = psy.tile([P, 64], F32, tag="warm_ps2", name="warm_ps2")
            nc.tensor.matmul(warm_ps2[:], warm_sb[:], warm_sb[:, :64])
            nc.scalar.activation(warm_act[:], warm_ps2[:], silu)
            for _ in range(48):
                nc.tensor.matmul(warm_ps[:], warm_sb[:], warm_sb[:, :64])

            for ci, (g0, w, e) in enumerate(chunks):
                if ci + 2 < len(chunks):
                    x_chunks.append(load_x_chunk(ci + 2))
                for ek in w_sched.get(ci, ()):
                    load_w_expert(ek)
                x_sb = x_chunks[ci]

                # phase 1: h_slice = silu(x @ W1_slice.T), F-major hs[f, tok]
                hs = hpool.tile([P, FTS, CHUNK], BF16, tag="hs", name="hs")[:, :, :w]
                for f in range(FTS):
                    ph = psh.tile([P, CHUNK], F32, tag="ph", name="ph")[:, :w]
                    for h in range(HT):
                        nc.tensor.matmul(
                            ph[:],
                            w1_sb[e][f][:, h, :],
                            x_sb[:, h, :],
                            start=(h == 0),
                            stop=(h == HT - 1),
                        )
                    nc.scalar.activation(hs[:, f, :], ph[:], silu)

                # gemm 2, H-major: py[h-tile, tok] += w2[f, h-tile].T @ hs[f].
                # Two half-H passes keep PSUM at 4 banks.
                y_sb = ypool.tile([P, HT, CHUNK], BF16, tag="y", name="y")[:, :, :w]
                for hg in range(2):
                    # f-outer over the group's four h-tiles, deferring each
                    # group's f=3 round: the last silu (f=3) then has ~12
                    # matmul slots of cover instead of 3, and the psum->sbuf
                    # copies spread across the f=3 round.
                    h0 = hg * (HT // 2)
                    pyh = [
                        psy.tile([P, CHUNK], F32, tag=f"pyh_{hh}", name=f"pyh_{hh}")[:, :w]
                        for hh in range(HT // 2)
                    ]
                    for f in range(FTS - 1):
                        for hh in range(HT // 2):
                            nc.tensor.matmul(
                                pyh[hh][:],
                                w2_sb[e][:, f, (h0 + hh) * P : (h0 + hh + 1) * P],
                                hs[:, f, :],
                                start=(f == 0),
                                stop=False,
                            )
                    for hh in range(HT // 2):
                        nc.tensor.matmul(
                            pyh[hh][:],
                            w2_sb[e][:, FTS - 1, (h0 + hh) * P : (h0 + hh + 1) * P],
                            hs[:, FTS - 1, :],
                            start=False,
                            stop=True,
                        )
                        nc.vector.tensor_scalar_mul(
                            y_sb[:, h0 + hh, :], pyh[hh][:], 1.0
                        )
                    # store each half-H group as soon as its copies land, so
                    # the kernel tail only waits on the last half (fanned
                    # over two rings on the final chunk to shorten it).
                    ysplit = 2 if (ci == len(chunks) - 1 and hg == 1) else 1
                    hstep = (HT // 2) // ysplit
                    for s in range(ysplit):
                        hb = h0 + s * hstep
                        nc.sync.dma_start(
                            y_d[:, hb : hb + hstep, g0 : g0 + w],
                            y_sb[:, hb : hb + hstep, :],
                        )

    nc.compile()
    return nc


def _get_program(plan_key):
    if plan_key not in _program_cache:
        _program_cache[plan_key] = _build_program(plan_key)
    return _program_cache[plan_key]


def _route(topk_e, topk_w):
    """Per-expert token indices and combined gate weights (duplicate top-k
    hits of the same expert are merged by summing their weights, matching
    the reference's repeated +=)."""
    idxs, wts = [], []
    for e in range(E):
        m = topk_e == e
        idx = np.nonzero(m.any(axis=1))[0]
        we = (topk_w.astype(np.float32) * m).sum(axis=1)[idx]
        idxs.append(idx)
        wts.append(we)
    return idxs, wts


def _ensure_device_healthy():
    """Probe the accelerator; if wedged (NRT unrecoverable), axon_reset it.
    Best-effort: silently skips when not running under the axon proxy."""
    try:
        import jax
        import jax.numpy as jnp
    except Exception:
        return
    for _ in range(3):
        try:
            a = jnp.ones((8, 8))
            float((a @ a).sum())
            return
        except Exception:
            try:
                import ctypes

                lib = ctypes.CDLL("/opt/axon/libaxon_pjrt.so")
                lib.axon_reset.restype = ctypes.c_int64
                lib.axon_reset()
            except Exception:
                return


def kernel(x, topk_e, topk_w, w1, w2):
    global LAST_RESULTS
    _ensure_device_healthy()
    x = np.ascontiguousarray(np.asarray(x), dtype=np.float32)
    topk_e = np.asarray(topk_e)
    topk_w = np.asarray(topk_w)
    w1 = np.asarray(w1, dtype=np.float32)
    w2 = np.asarray(w2, dtype=np.float32)
    S = x.shape[0]

    idxs, wts = _route(topk_e, topk_w)
    counts = [len(i) for i in idxs]
    T = sum(counts)
    chunks = _make_plan(counts)
    plan_key = (tuple(chunks), T)

    nc = _get_program(plan_key)

    bf = ml_dtypes.bfloat16

    # Shared x stream, p-major [P, HT, T]: xt[p, h, t] = x[pair[t], h*128+p].
    pair_idx = np.concatenate(idxs)
    xs = x[pair_idx].astype(bf)                              # [T, H]
    xt = np.ascontiguousarray(xs.T.reshape(HT, P, T).transpose(1, 0, 2))

    in_maps = []
    for j in range(N_CORES):
        f0 = j * FS
        # w1t[e, f, p, h, c] = w1[e, f0 + f*128 + c, h*128+p]
        w1t = np.ascontiguousarray(
            w1[:, f0 : f0 + FS, :].astype(bf)
            .reshape(E, FTS, P, HT, P).transpose(0, 1, 4, 3, 2)
        )
        # w2t[e, p, f, c] = w2[e, c, f0 + f*128 + p]
        w2t = np.ascontiguousarray(
            w2[:, :, f0 : f0 + FS].astype(bf)
            .reshape(E, H, FTS, P).transpose(0, 3, 2, 1)
        )
        in_maps.append({"xt": xt, "w1t": w1t, "w2t": w2t})

    res = run_bass_kernel_spmd(nc, in_maps, core_ids=list(range(N_CORES)))
    LAST_RESULTS = res

    # y_d is [P, HT, T] bf16 per core; sum cores, transpose to [T, H],
    # apply the combine weight, scatter-add per expert segment.
    ysum = np.zeros((P, HT, T), np.float32)
    for j in range(N_CORES):
        ysum += res.results[j]["y"].astype(np.float32)
    yt = ysum.transpose(2, 1, 0).reshape(T, H)
    wc_stream = np.concatenate(wts).astype(np.float32)
    yt *= wc_stream[:, None]

    y = np.zeros((S, H), np.float32)
    g0 = 0
    for e in range(E):
        L = counts[e]
        y[idxs[e]] += yt[g0 : g0 + L]
        g0 += L
    return y


# revision 48
# speedup vs baseline: 1.2319x; 1.0004x over previous
"""Trainium2 Bass kernel for nn_ExpertMLP (MoE routing).

Strategy (tensor-parallel over d_ff, host-side dispatch):
  - Every core processes ALL T token-expert pairs (the concatenation of
    each expert's distinct hit tokens, duplicate top-k hits merged with
    summed gate weight), but only a 512-wide slice of the FFN dim F:
    core j holds w1[e, j*512:(j+1)*512, :] and w2[e, :, j*512:(j+1)*512]
    for all 8 experts (16 MB bf16 -- same footprint as one full expert).
  - silu is elementwise, so the F-slice passes through it exactly:
        part_j = silu(x @ W1_j.T) @ W2_j.T          # [T, H]
        y(pair) = wc(pair) * sum_j part_j(pair)
    The host sums the 8 bf16 partials, applies wc, and scatter-adds per
    expert segment into the full [S, H] output.
  - Work per core is T * 2*2*H*(F/8) MACs -- perfectly balanced by
    construction (no capacity padding, no expert imbalance), ~6% less
    than the expert-parallel layout's max-expert padding, and with the
    H-major second GEMM there is no partial-tile waste at all:
    the PE stream is exactly 64*T cycles.

Device kernel layout (per core):
  The pair stream is cut into per-expert chunks of <=512 tokens, so every
  chunk uses a single expert's (resident) weight slice. Per chunk:
    phase 1:  for f in 4:  ph[128, w] += w1_sb[e][h, f-tile].T @ x[h]   (8 MMs)
              silu(ph) -> hs[:, f, :]   (ACT engine)
    gemm 2:   for h in 8:  pyh[128, w] += w2_sb[e][f, h-tile].T @ hs[f]  (4 MMs)
              copy pyh -> y_sb[:, h, :]  (DVE, f32->bf16)
    one [128, 8, w] bf16 DMA out per chunk (y stays H-major; the host
    transposes once after summing cores).
  All DMAs ride the sync-engine HWDGE queue in consumption order, one
  enqueue per chunk / per weight tensor (the host pre-lays p-major
  layouts so each load is a single 3D access pattern). Expert k's
  weights are enqueued three chunks before their first use. The scalar
  engine runs ONLY activations (its table is preloaded during the PE
  warm-up) so silu is never stuck behind DMA enqueues.

  SBUF/partition: w1 64KB + w2 64KB + x 32KB + hs 8KB + y 16KB ~= 184KB.
  PSUM banks: 2 (ph) + 4 (pyh) + 2 (warm) = 8.
"""

import numpy as np
import ml_dtypes

import concourse.bacc as bacc
import concourse.mybir as mybir
import concourse.tile as tile
from concourse.bass_utils import run_bass_kernel_spmd

P = 128
H = 1024
F = 4096
E = 8
N_CORES = 8
CHUNK = 512
FS = F // N_CORES   # 512: per-core f-slice
FTS = FS // P       # 4 f-tiles per core
HT = H // P         # 8

BF16 = mybir.dt.bfloat16
F32 = mybir.dt.float32

# Results of the most recent device run (BassKernelResults); lets a test
# harness read exec_time_ns / trace paths without changing kernel()'s API.
LAST_RESULTS = None

_program_cache = {}


def _make_plan(counts):
    """Cut the concatenated per-expert pair stream into single-expert
    chunks of <=512 tokens. Returns a list of (g0, w, e)."""
    chunks = []
    g0 = 0
    for e, L in enumerate(counts):
        if L == 0:
            continue
        n = -(-L // CHUNK)
        widths = [CHUNK] * (n - 1) + [L - CHUNK * (n - 1)]
        if n >= 2 and widths[-1] < P:
            widths[-2] = CHUNK // 2
            widths[-1] += CHUNK // 2
        for w in widths:
            chunks.append((g0, w, e))
            g0 += w
    return chunks


def _build_program(plan_key):
    chunks, T = plan_key
    nc = bacc.Bacc(None, name="expert_mlp_tp")

    xt_d = nc.dram_tensor("xt", (P, HT, T), BF16, kind="ExternalInput")
    # w1 is quarter-major: one fully-contiguous [P, HT*P] block per
    # (expert, f-tile), so each w1 DMA is 128 lines of 2KB and phase 1's
    # f=0 only depends on the first quarter's transfer.
    w1t_d = nc.dram_tensor("w1t", (E, FTS, P, HT, P), BF16, kind="ExternalInput")
    w2t_d = nc.dram_tensor("w2t", (E, P, FTS, H), BF16, kind="ExternalInput")
    y_d = nc.dram_tensor("y", (P, HT, T), BF16, kind="ExternalOutput")

    silu = mybir.ActivationFunctionType.Silu

    # Emit expert k's weight load 3 chunks ahead of its first chunk.
    first_chunk = {}
    for ci, (_, _, e) in enumerate(chunks):
        first_chunk.setdefault(e, ci)
    w_sched = {}
    head_experts = []
    for e, fc in first_chunk.items():
        at = fc - 3
        if at <= 0:
            head_experts.append(e)
        else:
            w_sched.setdefault(at, []).append(e)

    with tile.TileContext(nc) as tc:
        with (
            tc.tile_pool(name="wpool", bufs=1) as wpool,
            tc.tile_pool(name="xpool", bufs=4) as xpool,
            tc.tile_pool(name="hpool", bufs=2) as hpool,
            tc.tile_pool(name="ypool", bufs=2) as ypool,
            tc.tile_pool(name="spool", bufs=1) as spool,
            tc.tile_pool(name="psh", bufs=2, space="PSUM") as psh,
            tc.tile_pool(name="psy", bufs=1, space="PSUM") as psy,
        ):
            w1_sb = [
                [
                    wpool.tile([P, HT, P], BF16, tag=f"w1_{e}_{f}", name=f"w1_{e}_{f}")
                    for f in range(FTS)
                ]
                for e in range(E)
            ]
            w2_sb = [
                wpool.tile([P, FTS, H], BF16, tag=f"w2_{e}", name=f"w2_{e}")
                for e in range(E)
            ]

            def load_w_expert(e):
                for f in range(FTS):
                    nc.sync.dma_start(w1_sb[e][f][:], w1t_d[e, f])
                nc.sync.dma_start(w2_sb[e][:], w2t_d[e])

            def load_x_chunk(ci, split=1):
                # split>1 fans the load over several DMA rings -- used for
                # chunk 0, whose arrival gates the first real matmul.
                g0, w, _ = chunks[ci]
                t = xpool.tile([P, HT, CHUNK], BF16, tag="x", name="x")[:, :, :w]
                hstep = HT // split
                for s in range(split):
                    h0 = s * hstep
                    nc.sync.dma_start(
                        t[:, h0 : h0 + hstep, :],
                        xt_d[:, h0 : h0 + hstep, g0 : g0 + w],
                    )
                return t

            # Head DMAs, all on the sync queue (scalar-queue enqueues proved
            # slow and block the ACTs behind them). Enqueue instructions
            # cost ~0.6us of engine time each, so the order interleaves the
            # two tensors the first matmul actually waits on: the first
            # half of x chunk 0 and w1's first f-tile quarter. w2 is only
            # needed once gemm 2 starts and goes last.
            g0_0, w_0, _ = chunks[0]
            x_t0 = xpool.tile([P, HT, CHUNK], BF16, tag="x", name="x")[:, :, :w_0]
            hh = HT // 2
            nc.sync.dma_start(x_t0[:, :hh, :], xt_d[:, :hh, g0_0 : g0_0 + w_0])
            e0 = sorted(head_experts)[0] if head_experts else None
            if e0 is not None:
                nc.sync.dma_start(w1_sb[e0][0][:], w1t_d[e0, 0])
            nc.sync.dma_start(x_t0[:, hh:, :], xt_d[:, hh:, g0_0 : g0_0 + w_0])
            if e0 is not None:
                for f in range(1, FTS):
                    nc.sync.dma_start(w1_sb[e0][f][:], w1t_d[e0, f])
            x_chunks = [x_t0]
            if len(chunks) > 1:
                x_chunks.append(load_x_chunk(1))
            for e in sorted(head_experts):
                if e != e0:
                    for f in range(FTS):
                        nc.sync.dma_start(w1_sb[e][f][:], w1t_d[e, f])
                # split by H half: gemm 2's first half-group only needs
                # columns 0:512 (all f), so it can start off the first DMA.
                nc.sync.dma_start(w2_sb[e][:, :, 0 : H // 2], w2t_d[e][:, :, 0 : H // 2])
                nc.sync.dma_start(w2_sb[e][:, :, H // 2 :], w2t_d[e][:, :, H // 2 :])

            # PE pre-warm: dependency-free matmuls on a zeroed scratch tile
            # run while the head DMAs are in flight, so the HAM clock gate
            # is at 8/8 when the real matmuls start. One of them feeds a
            # throwaway silu so the ACT table loads off the critical path.
            warm_sb = spool.tile([P, P], BF16, tag="warm", name="warm_sb")
            warm_act = spool.tile([P, 64], BF16, tag="warm_act", name="warm_act")
            nc.gpsimd.memset(warm_sb[:], 0.0)
            warm_ps = psy.tile([P, 64], F32, tag="warm_ps", name="warm_ps")
            warm_ps2 = psy.tile([P, 64], F32, tag="warm_ps2", name="warm_ps2")
            nc.tensor.matmul(warm_ps2[:], warm_sb[:], warm_sb[:, :64])
            nc.scalar.activation(warm_act[:], warm_ps2[:], silu)
            for _ in range(72):
                nc.tensor.matmul(warm_ps[:], warm_sb[:], warm_sb[:, :64])

            for ci, (g0, w, e) in enumerate(chunks):
                if ci + 2 < len(chunks):
                    x_chunks.append(load_x_chunk(ci + 2))
                for ek in w_sched.get(ci, ()):
                    load_w_expert(ek)
                x_sb = x_chunks[ci]

                # phase 1: h_slice = silu(x @ W1_slice.T), F-major hs[f, tok]
                hs = hpool.tile([P, FTS, CHUNK], BF16, tag="hs", name="hs")[:, :, :w]
                for f in range(FTS):
                    ph = psh.tile([P, CHUNK], F32, tag="ph", name="ph")[:, :w]
                    for h in range(HT):
                        nc.tensor.matmul(
                            ph[:],
                            w1_sb[e][f][:, h, :],
                            x_sb[:, h, :],
                            start=(h == 0),
                            stop=(h == HT - 1),
                        )
                    nc.scalar.activation(hs[:, f, :], ph[:], silu)

                # gemm 2, H-major: py[h-tile, tok] += w2[f, h-tile].T @ hs[f].
                # Two half-H passes keep PSUM at 4 banks.
                y_sb = ypool.tile([P, HT, CHUNK], BF16, tag="y", name="y")[:, :, :w]
                for hg in range(2):
                    # f-outer over the group's four h-tiles, deferring each
                    # group's f=3 round: the last silu (f=3) then has ~12
                    # matmul slots of cover instead of 3, and the psum->sbuf
                    # copies spread across the f=3 round.
                    h0 = hg * (HT // 2)
                    pyh = [
                        psy.tile([P, CHUNK], F32, tag=f"pyh_{hh}", name=f"pyh_{hh}")[:, :w]
                        for hh in range(HT // 2)
                    ]
                    for f in range(FTS - 1):
                        for hh in range(HT // 2):
                            nc.tensor.matmul(
                                pyh[hh][:],
                                w2_sb[e][:, f, (h0 + hh) * P : (h0 + hh + 1) * P],
                                hs[:, f, :],
                                start=(f == 0),
                                stop=False,
                            )
                    for hh in range(HT // 2):
                        nc.tensor.matmul(
                            pyh[hh][:],
                            w2_sb[e][:, FTS - 1, (h0 + hh) * P : (h0 + hh + 1) * P],
                            hs[:, FTS - 1, :],
                            start=False,
                            stop=True,
                        )
                        nc.vector.tensor_scalar_mul(
                            y_sb[:, h0 + hh, :], pyh[hh][:], 1.0
                        )
                    # store each half-H group as soon as its copies land, so
                    # the kernel tail only waits on the last half (fanned
                    # over two rings on the final chunk to shorten it).
                    ysplit = 2 if (ci == len(chunks) - 1 and hg == 1) else 1
                    hstep = (HT // 2) // ysplit
                    for s in range(ysplit):
                        hb = h0 + s * hstep
                        nc.sync.dma_start(
                            y_d[:, hb : hb + hstep, g0 : g0 + w],
                            y_sb[:, hb : hb + hstep, :],
                        )

    nc.compile()
    return nc


def _get_program(plan_key):
    if plan_key not in _program_cache:
        _program_cache[plan_key] = _build_program(plan_key)
    return _program_cache[plan_key]


def _route(topk_e, topk_w):
    """Per-expert token indices and combined gate weights (duplicate top-k
    hits of the same expert are merged by summing their weights, matching
    the reference's repeated +=)."""
    idxs, wts = [], []
    for e in range(E):
        m = topk_e == e
        idx = np.nonzero(m.any(axis=1))[0]
        we = (topk_w.astype(np.float32) * m).sum(axis=1)[idx]
        idxs.append(idx)
        wts.append(we)
    return idxs, wts


def _ensure_device_healthy():
    """Probe the accelerator; if wedged (NRT unrecoverable), axon_reset it.
    Best-effort: silently skips when not running under the axon proxy."""
    try:
        import jax
        import jax.numpy as jnp
    except Exception:
        return
    for _ in range(3):
        try:
            a = jnp.ones((8, 8))
            float((a @ a).sum())
            return
        except Exception:
            try:
                import ctypes

                lib = ctypes.CDLL("/opt/axon/libaxon_pjrt.so")
                lib.axon_reset.restype = ctypes.c_int64
                lib.axon_reset()
            except Exception:
                return


def kernel(x, topk_e, topk_w, w1, w2):
    global LAST_RESULTS
    _ensure_device_healthy()
    x = np.ascontiguousarray(np.asarray(x), dtype=np.float32)
    topk_e = np.asarray(topk_e)
    topk_w = np.asarray(topk_w)
    w1 = np.asarray(w1, dtype=np.float32)
    w2 = np.asarray(w2, dtype=np.float32)
    S = x.shape[0]

    idxs, wts = _route(topk_e, topk_w)
    counts = [len(i) for i in idxs]
    T = sum(counts)
    chunks = _make_plan(counts)
    plan_key = (tuple(chunks), T)

    nc = _get_program(plan_key)

    bf = ml_dtypes.bfloat16

    # Shared x stream, p-major [P, HT, T]: xt[p, h, t] = x[pair[t], h*128+p].
    pair_idx = np.concatenate(idxs)
    xs = x[pair_idx].astype(bf)                              # [T, H]
    xt = np.ascontiguousarray(xs.T.reshape(HT, P, T).transpose(1, 0, 2))

    in_maps = []
    for j in range(N_CORES):
        f0 = j * FS
        # w1t[e, f, p, h, c] = w1[e, f0 + f*128 + c, h*128+p]
        w1t = np.ascontiguousarray(
            w1[:, f0 : f0 + FS, :].astype(bf)
            .reshape(E, FTS, P, HT, P).transpose(0, 1, 4, 3, 2)
        )
        # w2t[e, p, f, c] = w2[e, c, f0 + f*128 + p]
        w2t = np.ascontiguousarray(
            w2[:, :, f0 : f0 + FS].astype(bf)
            .reshape(E, H, FTS, P).transpose(0, 3, 2, 1)
        )
        in_maps.append({"xt": xt, "w1t": w1t, "w2t": w2t})

    res = run_bass_kernel_spmd(nc, in_maps, core_ids=list(range(N_CORES)))
    LAST_RESULTS = res

    # y_d is [P, HT, T] bf16 per core; sum cores, transpose to [T, H],
    # apply the combine weight, scatter-add per expert segment.
    ysum = np.zeros((P, HT, T), np.float32)
    for j in range(N_CORES):
        ysum += res.results[j]["y"].astype(np.float32)
    yt = ysum.transpose(2, 1, 0).reshape(T, H)
    wc_stream = np.concatenate(wts).astype(np.float32)
    yt *= wc_stream[:, None]

    y = np.zeros((S, H), np.float32)
    g0 = 0
    for e in range(E):
        L = counts[e]
        y[idxs[e]] += yt[g0 : g0 + L]
        g0 += L
    return y
